# revision 1
# baseline (speedup 1.0000x reference)
"""ChemProp directed-MPNN forward pass on 8 Trainium2 NeuronCores (Bass/Tile).

Self-contained kernel. Strategy: data-parallel over graphs with duplicated
rev-closure edge compute (no cross-core h exchange); only the two small
node-sum arrays are AllGathered (bf16). All gathers are local int16-windowed
dma_gathers (transposed 512-row calls feed the TensorEngine directly);
segment sums run as selection-matrix matmuls with PSUM-resident group
accumulators. kernel(**inputs) -> np.float32 [5000, 256].
"""
import sys
sys.path.insert(0, "/opt/trn_rl_repo")
import numpy as np
import ml_dtypes
import concourse.bass as bass
import concourse.bacc as bacc
import concourse.mybir as mybir
import concourse.tile as tile
from concourse.library_config import mlp

N_NODES = 160000
N_EDGES = 640000
NUM_GRAPHS = 5000

bf16 = ml_dtypes.bfloat16

H = 256        # hidden
XD = 128       # node feature dim
ED = 64        # edge feature dim


def _ceil(a, b):
    return -(-a // b)


def _rup(a, b):
    return _ceil(a, b) * b


def preprocess(inputs, G, NC=8, WIN=32768, CHUNK=2048):
    """Host-side index preprocessing. Returns (in_maps, meta, post)."""
    TILE = 128
    x = np.asarray(inputs["x"], np.float32)
    ei = np.asarray(inputs["edge_index"]).astype(np.int64)
    rev = np.asarray(inputs["revedge_index"]).astype(np.int64)
    ea = np.asarray(inputs["edge_attr"], np.float32)
    batch = np.asarray(inputs["batch"]).astype(np.int64)
    W1m = np.asarray(inputs["W1"], np.float32)
    W2m = np.asarray(inputs["W2"], np.float32)
    W3m = np.asarray(inputs["W3"], np.float32)
    b3 = np.asarray(inputs["b3"], np.float32)
    N = x.shape[0]
    E = ei.shape[1]
    src, dst = ei[0], ei[1]

    # ---- 1. graph -> core split by in-edge counts -------------------------
    edge_g = batch[dst]
    epg = np.bincount(edge_g, minlength=G)
    cum = np.concatenate([[0], np.cumsum(epg)])
    bounds = [int(np.searchsorted(cum, c * E / NC)) for c in range(NC + 1)]
    bounds[0], bounds[NC] = 0, G
    for c in range(1, NC):  # keep monotone
        bounds[c] = min(max(bounds[c], bounds[c - 1]), G)
    gr = [(bounds[c], bounds[c + 1]) for c in range(NC)]
    # node ranges: batch is sorted; node n belongs to graph batch[n]
    nb = np.searchsorted(batch, bounds)  # [NC+1]
    core_of_node = np.zeros(N, np.int64)
    for c in range(NC):
        core_of_node[nb[c]:nb[c + 1]] = c

    # ---- 2. node slot space (graph-group padded, uniform boundaries) -----
    ngraphs = [gr[c][1] - gr[c][0] for c in range(NC)]
    GPAD = _rup(max(max(ngraphs), 1), TILE)
    NGG = GPAD // TILE
    # per (core, gg): node count
    ggcnt = np.zeros((NC, NGG), np.int64)
    for c in range(NC):
        glo, ghi = gr[c]
        for gg in range(NGG):
            g0, g1 = glo + gg * TILE, min(glo + (gg + 1) * TILE, ghi)
            if g0 >= ghi:
                continue
            n0 = np.searchsorted(batch, g0)
            n1 = np.searchsorted(batch, g1)
            ggcnt[c, gg] = n1 - n0
    ggpad = np.maximum(_rup(ggcnt.max(axis=0), TILE), TILE)  # uniform per gg
    ggoff = np.concatenate([[0], np.cumsum(ggpad)])
    S = int(ggoff[-1])
    NG = S // TILE
    NSROWS = NC * S
    NSW = _ceil(NSROWS, WIN)

    node_slot = np.full(N, -1, np.int64)
    for c in range(NC):
        glo, ghi = gr[c]
        for gg in range(NGG):
            g0, g1 = glo + gg * TILE, min(glo + (gg + 1) * TILE, ghi)
            if g0 >= ghi:
                continue
            n0 = np.searchsorted(batch, g0)
            n1 = np.searchsorted(batch, g1)
            node_slot[n0:n1] = ggoff[gg] + np.arange(n1 - n0)
    node_nsrow = core_of_node * S + node_slot  # global ns row per node
    sw_of_edge = node_nsrow[src] // WIN        # s-gather window key per edge

    # tile -> graph-group map (uniform)
    tile_gg = np.zeros(NG, np.int64)
    for gg in range(NGG):
        tile_gg[ggoff[gg] // TILE: ggoff[gg + 1] // TILE] = gg

    # ---- 3. per-core worklists -------------------------------------------
    cores = []
    for c in range(NC):
        E_c = np.nonzero(core_of_node[dst] == c)[0]
        R = rev[E_c]
        W1s = np.union1d(E_c, R)
        D = rev[W1s]
        W0s = np.union1d(W1s, D)
        t = np.full(E, -1, np.int8)
        t[W0s] = 2
        t[W1s] = 1
        t[E_c] = 0
        cores.append(dict(E_c=E_c, W0=W0s, t=t))

    # ---- 4. classes, buckets, uniform padded layout ----------------------
    NCLS = 3 * NSW
    cls_of = []  # per core: dict edge -> class (only for W0 members)
    raw_cls = np.zeros((NC, NCLS), np.int64)
    for c in range(NC):
        W0s = cores[c]["W0"]
        t = cores[c]["t"][W0s].astype(np.int64)
        k = t * NSW + sw_of_edge[W0s]
        cls_arr = np.full(E, -1, np.int64)
        cls_arr[W0s] = k
        cls_of.append(cls_arr)
        raw_cls[c] = np.bincount(k, minlength=NCLS)

    SLACK = (NCLS + 2) * TILE
    cls_ub = _rup(raw_cls.max(axis=0), TILE) + SLACK
    assert (cls_ub <= WIN).all(), "class exceeds int16 window; need finer split"

    # super-windows: group consecutive classes, ub-total <= WIN
    supwin_of_cls = np.zeros(NCLS, np.int64)
    sw_id, acc = 0, 0
    for k in range(NCLS):
        if (acc + cls_ub[k] > WIN and acc > 0) or (k % NSW == 0 and k > 0):
            sw_id += 1
            acc = 0
        supwin_of_cls[k] = sw_id
        acc += cls_ub[k]
    NSUP = sw_id + 1
    # supwins never straddle t boundaries (classes are t-major); verify:
    for s in range(NSUP):
        ks = np.nonzero(supwin_of_cls == s)[0]
        assert len(set(ks // NSW)) == 1, "supwin straddles t boundary"

    # bucket sizes: class (t<2) x gw0(rev) supwin
    raw_bkt = np.zeros((NC, 2 * NSW, NSUP), np.int64)
    for c in range(NC):
        W0s = cores[c]["W0"]
        t = cores[c]["t"][W0s]
        m = t < 2
        e01 = W0s[m]
        k01 = cls_of[c][e01]
        gw = supwin_of_cls[cls_of[c][rev[e01]]]
        np.add.at(raw_bkt[c], (k01, gw), 1)
    bktpad = _rup(raw_bkt.max(axis=0), TILE)  # [2*NSW, NSUP]

    cls_final = np.zeros(NCLS, np.int64)
    cls_final[: 2 * NSW] = bktpad.sum(axis=1)
    cls_final[2 * NSW:] = _rup(raw_cls.max(axis=0)[2 * NSW:], TILE)
    cls_final = np.maximum(cls_final, TILE)
    assert (cls_final <= cls_ub).all()
    cls_off = np.concatenate([[0], np.cumsum(cls_final)])
    O0pad = int(cls_off[NCLS])
    O1E = int(cls_off[NSW])        # end of t=0 classes
    O1pad = int(cls_off[2 * NSW])  # end of t=1 classes
    supwin_base = np.array([int(cls_off[np.nonzero(supwin_of_cls == s)[0][0]])
                            for s in range(NSUP)])
    for s in range(NSUP):
        ks = np.nonzero(supwin_of_cls == s)[0]
        span = cls_off[ks[-1] + 1] - supwin_base[s]
        assert span <= WIN, f"supwin {s} span {span}"
    bkt_off = np.zeros((2 * NSW, NSUP), np.int64)
    for k in range(2 * NSW):
        bkt_off[k] = cls_off[k] + np.concatenate([[0], np.cumsum(bktpad[k])[:-1]])

    # ---- 5. per-core O0 slot assignment ----------------------------------
    slot_edge = np.full((NC, O0pad), -1, np.int64)  # edge id per slot (-1 pad)
    pos0 = np.full((NC, E), -1, np.int64)           # edge -> O0 slot
    for c in range(NC):
        W0s = cores[c]["W0"]
        t = cores[c]["t"][W0s]
        k = cls_of[c][W0s]
        key_src = node_nsrow[src[W0s]]
        # t<2: bucket by gw0; t=2: single block
        gw = np.where(t < 2, supwin_of_cls[cls_of[c][rev[W0s]]], 0)
        order = np.lexsort((key_src, gw, k))
        W0o, ko, gwo = W0s[order], k[order], gw[order]
        # rank within (class,bucket)
        # vectorized rank within (class,bucket)
        if len(W0o):
            keypair = ko * (NSUP + 1) + np.where(ko < 2 * NSW, gwo, 0)
            newgrp = np.concatenate([[True], keypair[1:] != keypair[:-1]])
            grp_id = np.cumsum(newgrp) - 1
            grp_first = np.nonzero(newgrp)[0]
            rank = np.arange(len(W0o)) - grp_first[grp_id]
            base = np.where(ko < 2 * NSW, bkt_off[np.minimum(ko, 2 * NSW - 1), gwo],
                            cls_off[ko])
            slots = base + rank
            slot_edge[c, slots] = W0o
            pos0[c, W0o] = slots

    # ---- 6. O2 layout (iter2 over E_c) -----------------------------------
    raw2 = np.zeros((NC, NSW, NSUP), np.int64)
    for c in range(NC):
        E_c = cores[c]["E_c"]
        swk = sw_of_edge[E_c]
        gw1 = supwin_of_cls[cls_of[c][rev[E_c]]]
        np.add.at(raw2, (np.full(len(E_c), c), swk, gw1), 1)
    pad2 = _rup(raw2.max(axis=0), TILE)  # [NSW, NSUP]
    off2 = np.concatenate([[0], np.cumsum(pad2.reshape(-1))]).reshape(-1)
    m2pad = int(off2[-1])
    slot2_edge = np.full((NC, m2pad), -1, np.int64)
    pos2 = np.full((NC, E), -1, np.int64)
    for c in range(NC):
        E_c = cores[c]["E_c"]
        swk = sw_of_edge[E_c]
        gw1 = supwin_of_cls[cls_of[c][rev[E_c]]]
        order = np.lexsort((node_nsrow[src[E_c]], gw1, swk))
        Eo, swo, gwo = E_c[order], swk[order], gw1[order]
        keypair = swo * NSUP + gwo
        newgrp = np.concatenate([[True], keypair[1:] != keypair[:-1]])
        grp_first = np.nonzero(newgrp)[0]
        rank = np.arange(len(Eo)) - grp_first[np.cumsum(newgrp) - 1]
        slots = off2[keypair] + rank
        slot2_edge[c, slots] = Eo
        pos2[c, Eo] = slots

    # ---- 7. fold01 (ns0/ns1 over h[E_c]) ---------------------------------
    # group-block-major: for each block of BG groups, sweep all subpasses with
    # PSUM-resident per-group accumulators (no SBUF accumulation needed).
    BG = 4
    rawF = np.zeros((NC, NSW, NG), np.int64)
    for c in range(NC):
        E_c = cores[c]["E_c"]
        w = sw_of_edge[E_c]
        g = node_slot[dst[E_c]] // TILE
        np.add.at(rawF, (np.full(len(E_c), c), w, g), 1)
    ntilesF = _ceil(rawF.max(axis=0), TILE)      # [NSW, NG]
    ntilesF[0] = np.maximum(ntilesF[0], 1)       # force first-touch for all g

    def _fold_layout(ntiles, nw):
        tiles = []
        calls = []           # (slot0, n, w) -- src_base added later
        tile_off_wg = {}
        off = 0
        for gb in range(0, NG, BG):
            gs_ = list(range(gb, min(gb + BG, NG)))
            lastw = {g: max((w for w in range(nw) if ntiles[w, g] > 0), default=0)
                     for g in gs_}
            for w in range(nw):
                call_t0 = len(tiles)
                for g in gs_:
                    nt = int(ntiles[w, g])
                    if nt == 0:
                        continue
                    tile_off_wg[(w, g)] = off
                    for ti in range(nt):
                        tiles.append(dict(w=w, g=g,
                                          start=(w == 0 and ti == 0),
                                          stop=(w == lastw[g] and ti == nt - 1)))
                        off += TILE
                ncall = (len(tiles) - call_t0) * TILE
                if ncall > 0:
                    calls.append(dict(slot0=off - ncall, n=ncall, w=w))
        return tiles, calls, tile_off_wg, off

    fold_tiles, fold_calls_raw, tile_off_wg, foldF_slots = _fold_layout(ntilesF, NSW)
    foldF_edge = np.full((NC, foldF_slots), -1, np.int64)
    for c in range(NC):
        E_c = cores[c]["E_c"]
        w = sw_of_edge[E_c]
        g = node_slot[dst[E_c]] // TILE
        order = np.lexsort((node_slot[dst[E_c]], g, w))
        Eo, wo, go = E_c[order], w[order], g[order]
        keypair = wo * NG + go
        newgrp = np.concatenate([[True], keypair[1:] != keypair[:-1]])
        grp_first = np.nonzero(newgrp)[0]
        rank = np.arange(len(Eo)) - grp_first[np.cumsum(newgrp) - 1]
        base = np.array([tile_off_wg[(int(a), int(b))] for a, b in zip(wo[grp_first], go[grp_first])])
        slots = base[np.cumsum(newgrp) - 1] + rank
        foldF_edge[c, slots] = Eo

    # ---- 8. fold2 (ns2 over h2) ------------------------------------------
    NSUP2 = _ceil(m2pad, WIN)
    rawF2 = np.zeros((NC, NSUP2, NG), np.int64)
    for c in range(NC):
        E_c = cores[c]["E_c"]
        w = pos2[c, E_c] // WIN
        g = node_slot[dst[E_c]] // TILE
        np.add.at(rawF2, (np.full(len(E_c), c), w, g), 1)
    ntilesF2 = _ceil(rawF2.max(axis=0), TILE)
    ntilesF2[0] = np.maximum(ntilesF2[0], 1)
    fold2_tiles, fold2_calls_raw, tile_off_wg2, foldF2_slots = _fold_layout(ntilesF2, NSUP2)
    foldF2_edge = np.full((NC, foldF2_slots), -1, np.int64)
    for c in range(NC):
        E_c = cores[c]["E_c"]
        w = pos2[c, E_c] // WIN
        g = node_slot[dst[E_c]] // TILE
        order = np.lexsort((node_slot[dst[E_c]], g, w))
        Eo, wo, go = E_c[order], w[order], g[order]
        keypair = wo * NG + go
        newgrp = np.concatenate([[True], keypair[1:] != keypair[:-1]])
        grp_first = np.nonzero(newgrp)[0]
        rank = np.arange(len(Eo)) - grp_first[np.cumsum(newgrp) - 1]
        base = np.array([tile_off_wg2[(int(a), int(b))] for a, b in zip(wo[grp_first], go[grp_first])])
        slots = base[np.cumsum(newgrp) - 1] + rank
        foldF2_edge[c, slots] = Eo

    # ---- 9. call tables ---------------------------------------------------
    # L1 (P-gather): per class, chunks
    l1_calls = []
    for k in range(NCLS):
        n = int(cls_final[k])
        base = k % NSW  # class sw
        s0 = int(cls_off[k])
        for o in range(0, n, CHUNK):
            l1_calls.append(dict(slot0=s0 + o, n=min(CHUNK, n - o),
                                 p_base=int((k % NSW) * WIN)))
    # iter1: per (class t<2, gw) bucket
    def region_of(slotbase):
        if slotbase < O1E:
            return "E", slotbase
        if slotbase < O1pad:
            return "R", slotbase - O1E
        return "D", slotbase - O1pad

    it1_calls = []
    for k in range(2 * NSW):
        for wsup in range(NSUP):
            n = int(bktpad[k, wsup])
            if n == 0:
                continue
            s0 = int(bkt_off[k, wsup])
            greg, gb = region_of(int(supwin_base[wsup]))
            for o in range(0, n, CHUNK):
                it1_calls.append(dict(
                    slot0=s0 + o, n=min(CHUNK, n - o),
                    s_base=int((k % NSW) * WIN),
                    g_reg=greg, g_base=int(gb),
                    t=k // NSW))
    # iter2: per (sw, gw1) bucket
    it2_calls = []
    for swk in range(NSW):
        s0_supwin = supwin_of_cls[swk]  # class (0, swk) supwin
        hreg, hb = region_of(int(supwin_base[s0_supwin]))
        assert hreg == "E"
        for wsup in range(NSUP):
            n = int(pad2[swk, wsup])
            if n == 0:
                continue
            s0 = int(off2[swk * NSUP + wsup])
            greg, gb = region_of(int(supwin_base[wsup]))
            for o in range(0, n, CHUNK):
                it2_calls.append(dict(
                    slot0=s0 + o, n=min(CHUNK, n - o),
                    s_base=int(swk * WIN),
                    g_reg=greg, g_base=int(gb),
                    h_base=int(hb)))
    # fold01/fold2 calls: one per (group-block, w); attach src bases; split at CHUNK
    fold_calls = []
    for cl in fold_calls_raw:
        n0 = cl["n"]
        for o in range(0, n0, CHUNK):
            fold_calls.append(dict(slot0=cl["slot0"] + o, n=min(CHUNK, n0 - o),
                                   src_base=int(cls_off[cl["w"]])))
    fold2_calls = []
    for cl in fold2_calls_raw:
        n0 = cl["n"]
        for o in range(0, n0, CHUNK):
            fold2_calls.append(dict(slot0=cl["slot0"] + o, n=min(CHUNK, n0 - o),
                                    src_base=int(cl["w"] * WIN)))

    # ---- 10. stage per-core arrays ---------------------------------------
    def wrap_idx(flat):
        a = np.asarray(flat, np.int16).reshape(-1, 16).T  # [16, L/16]
        return np.tile(a, (8, 1))

    xbf = x.astype(bf16)
    xg_T = np.zeros((128, NSROWS), bf16)
    for c in range(NC):
        nlo, nhi = nb[c], nb[c + 1]
        ns_slots = c * S + node_slot[nlo:nhi]
        xg_T[:, ns_slots] = xbf[nlo:nhi].T
    eabf = ea.astype(bf16)

    iota = np.tile(np.arange(128, dtype=np.float32).astype(bf16)[None, :], (128, 1))
    ident = np.eye(128, dtype=np.float32).astype(bf16)
    w1e = W1m[XD:].astype(bf16)                       # [64, 256]
    w1x = W1m[:XD].astype(bf16)                       # [128, 256]
    w2 = W2m.reshape(2, 128, H).transpose(1, 0, 2).astype(bf16)   # [128,2,256] chunk k = [:,k,:] = W2[k*128:(k+1)*128]
    w3x = W3m[:XD].astype(bf16)
    w3v = W3m[XD:].reshape(2, 128, H).transpose(1, 0, 2).astype(bf16)
    b3row = b3.astype(bf16)[None, :]                  # [1, 256]
    ones1 = np.ones((1, 128), bf16)

    in_maps = []
    post = dict(gr=gr, S=S, GPAD=GPAD)
    for c in range(NC):
        se = slot_edge[c]
        valid = se >= 0
        sev = np.where(valid, se, 0)
        # eaT [64, O0pad]
        eaT = np.where(valid[None, :], eabf[sev].T, bf16(0))
        # L1 P-gather idx: nsrow(src) - swbase per class
        idxP = np.zeros(O0pad, np.int64)
        swslot = np.zeros(O0pad, np.int64)
        for k in range(NCLS):
            swslot[cls_off[k]:cls_off[k + 1]] = (k % NSW) * WIN
        idxP = np.where(valid, node_nsrow[src[sev]] - swslot, 0)
        idxP = np.clip(idxP, 0, WIN - 1)
        # iter1 idx (over O1 slots)
        n1 = O1pad
        se1 = se[:n1]
        v1 = se1 >= 0
        se1v = np.where(v1, se1, 0)
        idx1s = np.where(v1, node_nsrow[src[se1v]] - swslot[:n1], 0)
        gsup = np.zeros(n1, np.int64)
        for k in range(2 * NSW):
            for wsup in range(NSUP):
                a, b = bkt_off[k, wsup], bkt_off[k, wsup] + bktpad[k, wsup]
                gsup[a:b] = supwin_base[wsup]
        idx1g = np.where(v1, pos0[c, rev[se1v]] - gsup, 0)
        idx1s = np.clip(idx1s, 0, WIN - 1)
        idx1g = np.clip(idx1g, 0, WIN - 1)
        # iter2 idx
        se2 = slot2_edge[c]
        v2 = se2 >= 0
        se2v = np.where(v2, se2, 0)
        sw2slot = np.zeros(m2pad, np.int64)
        g2sup = np.zeros(m2pad, np.int64)
        h2base = np.zeros(m2pad, np.int64)
        for swk in range(NSW):
            for wsup in range(NSUP):
                a = off2[swk * NSUP + wsup]
                b = a + pad2[swk, wsup]
                sw2slot[a:b] = swk * WIN
                g2sup[a:b] = supwin_base[wsup]
                h2base[a:b] = supwin_base[supwin_of_cls[swk]]
        idx2s = np.where(v2, node_nsrow[src[se2v]] - sw2slot, 0)
        idx2g = np.where(v2, pos0[c, rev[se2v]] - g2sup, 0)
        idx2h = np.where(v2, pos0[c, se2v] - h2base, 0)
        idx2s = np.clip(idx2s, 0, WIN - 1)
        idx2g = np.clip(idx2g, 0, WIN - 1)
        idx2h = np.clip(idx2h, 0, WIN - 1)
        # fold01
        fe = foldF_edge[c]
        vf = fe >= 0
        fev = np.where(vf, fe, 0)
        fbase = np.zeros(foldF_slots, np.int64)
        for ti, tm in enumerate(fold_tiles):
            fbase[ti * TILE:(ti + 1) * TILE] = cls_off[tm["w"]]
        idxF = np.where(vf, pos0[c, fev] - fbase, 0)
        idxF = np.clip(idxF, 0, WIN - 1)
        dF = np.where(vf, (node_slot[dst[fev]] % TILE).astype(np.float32), -1.0)
        dF_t = dF.reshape(-1, TILE).T.astype(bf16)  # [128, ntiles]
        # fold2
        fe2 = foldF2_edge[c]
        vf2 = fe2 >= 0
        fev2 = np.where(vf2, fe2, 0)
        fbase2 = np.zeros(foldF2_slots, np.int64)
        for ti, tm in enumerate(fold2_tiles):
            fbase2[ti * TILE:(ti + 1) * TILE] = tm["w"] * WIN
        idxF2 = np.where(vf2, pos2[c, fev2] - fbase2, 0)
        idxF2 = np.clip(idxF2, 0, WIN - 1)
        dF2 = np.where(vf2, (node_slot[dst[fev2]] % TILE).astype(np.float32), -1.0)
        dF2_t = dF2.reshape(-1, TILE).T.astype(bf16)
        # L3
        nlo, nhi = nb[c], nb[c + 1]
        xT_c = np.zeros((128, S), bf16)
        xT_c[:, node_slot[nlo:nhi]] = xbf[nlo:nhi].T
        dPool = np.full(S, -1.0, np.float32)
        glo = gr[c][0]
        lg = batch[nlo:nhi] - glo
        dPool[node_slot[nlo:nhi]] = (lg % TILE).astype(np.float32)
        dPool_t = dPool.reshape(-1, TILE).T.astype(bf16)  # [128, NG]

        in_maps.append(dict(
            xg_T=xg_T, xT_c=xT_c, eaT=np.ascontiguousarray(eaT),
            idxP=wrap_idx(idxP), idx1s=wrap_idx(idx1s), idx1g=wrap_idx(idx1g),
            idx2s=wrap_idx(idx2s), idx2g=wrap_idx(idx2g), idx2h=wrap_idx(idx2h),
            idxF=wrap_idx(idxF), idxF2=wrap_idx(idxF2),
            dF=np.ascontiguousarray(dF_t), dF2=np.ascontiguousarray(dF2_t),
            dPool=np.ascontiguousarray(dPool_t),
            w1e=w1e, w1x=w1x, w2=np.ascontiguousarray(w2),
            w3x=w3x, w3v=np.ascontiguousarray(w3v), b3row=b3row, ones1=ones1,
            iota=iota, ident=ident,
        ))

    meta = dict(
        NC=NC, WIN=WIN, CHUNK=CHUNK, S=S, NG=NG, GPAD=GPAD, NGG=NGG,
        NSROWS=NSROWS, NSW=NSW, NSUP=NSUP, NSUP2=NSUP2,
        O0pad=O0pad, O1E=O1E, O1pad=O1pad, m2pad=m2pad,
        foldF_slots=foldF_slots, foldF2_slots=foldF2_slots,
        l1_calls=l1_calls, it1_calls=it1_calls, it2_calls=it2_calls,
        fold_calls=fold_calls, fold2_calls=fold2_calls,
        fold_tiles=fold_tiles, fold2_tiles=fold2_tiles,
        tile_gg=tile_gg.tolist(),
        cls_off=cls_off.tolist(), supwin_base=supwin_base.tolist(),
    )
    return in_maps, meta, post





dt = mybir.dt
Alu = mybir.AluOpType
Act = mybir.ActivationFunctionType
H = 256
TILE = 128


def build(meta, repeat=1, timing_mode=False, stages=None):
    def on(name):
        return stages is None or name in stages

    NC = meta["NC"]
    WIN, CHUNK = meta["WIN"], meta["CHUNK"]
    S, NG, GPAD, NGG = meta["S"], meta["NG"], meta["GPAD"], meta["NGG"]
    NSROWS = meta["NSROWS"]
    O0pad, O1E, O1pad, m2pad = meta["O0pad"], meta["O1E"], meta["O1pad"], meta["m2pad"]
    nFt = len(meta["fold_tiles"])
    nF2t = len(meta["fold2_tiles"])
    tile_gg = meta["tile_gg"]

    nc = bacc.Bacc(None, target_bir_lowering=False)

    def din(name, shape, d):
        return nc.declare_dram_parameter(name, list(shape), d, isOutput=False)

    xg_T = din("xg_T", [128, NSROWS], dt.bfloat16)
    xT_c = din("xT_c", [128, S], dt.bfloat16)
    eaT = din("eaT", [64, O0pad], dt.bfloat16)
    idxP = din("idxP", [128, O0pad // 16], dt.int16)
    idx1s = din("idx1s", [128, O1pad // 16], dt.int16)
    idx1g = din("idx1g", [128, O1pad // 16], dt.int16)
    idx2s = din("idx2s", [128, m2pad // 16], dt.int16)
    idx2g = din("idx2g", [128, m2pad // 16], dt.int16)
    idx2h = din("idx2h", [128, m2pad // 16], dt.int16)
    idxF = din("idxF", [128, meta["foldF_slots"] // 16], dt.int16)
    idxF2 = din("idxF2", [128, meta["foldF2_slots"] // 16], dt.int16)
    dF = din("dF", [128, nFt], dt.bfloat16)
    dF2 = din("dF2", [128, nF2t], dt.bfloat16)
    dPool = din("dPool", [128, NG], dt.bfloat16)
    w1e = din("w1e", [64, H], dt.bfloat16)
    w1x = din("w1x", [128, H], dt.bfloat16)
    w2 = din("w2", [128, 2, H], dt.bfloat16)
    w3x = din("w3x", [128, H], dt.bfloat16)
    w3v = din("w3v", [128, 2, H], dt.bfloat16)
    b3row = din("b3row", [1, H], dt.bfloat16)
    ones1 = din("ones1", [1, 128], dt.bfloat16)
    iota = din("iota", [128, 128], dt.bfloat16)
    ident = din("ident", [128, 128], dt.bfloat16)
    out = nc.declare_dram_parameter("out", [GPAD, H], dt.float32, isOutput=True)

    def reg_tensor_base(slot):
        if slot < O1E:
            return "E", slot
        if slot < O1pad:
            return "R", slot - O1E
        return "D", slot - O1pad

    with tile.TileContext(nc) as tc:
        with (
            tc.tile_pool(name="dram", bufs=1, space="DRAM") as dram,
            tc.tile_pool(name="const", bufs=1) as cpool,
            tc.tile_pool(name="idx", bufs=4) as ipool,
            tc.tile_pool(name="ring", bufs=2) as ring,
            tc.tile_pool(name="stage", bufs=2) as stg,
            tc.tile_pool(name="mwork", bufs=3) as mw,
            tc.tile_pool(name="psA", bufs=2, space="PSUM") as psA,
            tc.tile_pool(name="psT", bufs=2, space="PSUM") as psT,
            tc.tile_pool(name="psF", bufs=1, space="PSUM") as psF,
        ):
            nc.gpsimd.load_library(mlp)

            # constants to SBUF
            w1e_t = cpool.tile([64, H], dt.bfloat16)
            nc.sync.dma_start(out=w1e_t[:], in_=w1e[:])
            w1x_t = cpool.tile([128, H], dt.bfloat16)
            nc.sync.dma_start(out=w1x_t[:], in_=w1x[:])
            w2_t = cpool.tile([128, 2 * H], dt.bfloat16)
            nc.sync.dma_start(out=w2_t[:], in_=w2[:].rearrange("p a b -> p (a b)"))
            w3x_t = cpool.tile([128, H], dt.bfloat16)
            nc.sync.dma_start(out=w3x_t[:], in_=w3x[:])
            w3v_t = cpool.tile([128, 2 * H], dt.bfloat16)
            nc.sync.dma_start(out=w3v_t[:], in_=w3v[:].rearrange("p a b -> p (a b)"))
            b3_t = cpool.tile([1, H], dt.bfloat16)
            nc.sync.dma_start(out=b3_t[:], in_=b3row[:])
            ones_t = cpool.tile([1, 128], dt.bfloat16)
            nc.sync.dma_start(out=ones_t[:], in_=ones1[:])
            iota_t = cpool.tile([128, 128], dt.bfloat16)
            nc.sync.dma_start(out=iota_t[:], in_=iota[:])
            ident_t = cpool.tile([128, 128], dt.bfloat16)
            nc.sync.dma_start(out=ident_t[:], in_=ident[:])
            dF_t = cpool.tile([128, nFt], dt.bfloat16)
            nc.sync.dma_start(out=dF_t[:], in_=dF[:])
            dF2_t = cpool.tile([128, nF2t], dt.bfloat16)
            nc.sync.dma_start(out=dF2_t[:], in_=dF2[:])
            dPool_t = cpool.tile([128, NG], dt.bfloat16)
            nc.sync.dma_start(out=dPool_t[:], in_=dPool[:])

            for _rep in range(repeat):
                # internal DRAM buffers
                P = dram.tile([NSROWS, H], dt.bfloat16)
                h0E = dram.tile([O1E, H], dt.bfloat16)
                h0R = dram.tile([max(O1pad - O1E, TILE), H], dt.bfloat16)
                h0D = dram.tile([max(O0pad - O1pad, TILE), H], dt.bfloat16)
                h1E = dram.tile([O1E, H], dt.bfloat16)
                h1R = dram.tile([max(O1pad - O1E, TILE), H], dt.bfloat16)
                h2d = dram.tile([m2pad, H], dt.bfloat16)
                ns2d = dram.tile([S, H], dt.bfloat16)
                ns0s = dram.tile([S, H], dt.bfloat16)
                ns1s = dram.tile([S, H], dt.bfloat16)
                ns0f = dram.tile([NSROWS, H], dt.bfloat16, addr_space="Shared")
                ns1f = dram.tile([NSROWS, H], dt.bfloat16, addr_space="Shared")
                hreg0 = {"E": (h0E, O1E), "R": (h0R, O1pad - O1E), "D": (h0D, O0pad - O1pad)}
                hreg1 = {"E": (h1E, O1E), "R": (h1R, O1pad - O1E)}

                # ---- P pass: P = x @ W1x over ns-slot space --------------------
                PC = 2048
                for b in range(0, NSROWS, PC) if on("P") else []:
                    n = min(PC, NSROWS - b)
                    xc = ring.tile([128, PC], dt.bfloat16, tag="xc")
                    nc.sync.dma_start(out=xc[:, :n], in_=xg_T[:, b:b + n])
                    pstage = stg.tile([128, (PC // 128) * H], dt.bfloat16, tag="hst")
                    for t0 in range(0, n // 128, 2):
                        pn = min(2, n // 128 - t0)
                        ps = psA.tile([128, 2 * H], dt.float32, tag="main")
                        for j in range(pn):
                            t = t0 + j
                            nc.tensor.matmul(out=ps[:, j * H:(j + 1) * H], lhsT=xc[:, t * 128:(t + 1) * 128],
                                             rhs=w1x_t[:], start=True, stop=True)
                        nc.scalar.activation(pstage[:, t0 * H:(t0 + pn) * H], ps[:, :pn * H], Act.Copy)
                    nc.sync.dma_start(
                        out=P[b:b + n, :].rearrange("(t p) d -> p t d", p=128),
                        in_=pstage[:, :(n // 128) * H].rearrange("p (t d) -> p t d", d=H))

                # ---- L1: h0 = relu(P[src] + eaT.T @ W1e) -----------------------
                def l1_do(call):
                    s0, n, pb = call["slot0"], call["n"], call["p_base"]
                    nt = n // 128
                    it = ipool.tile([128, CHUNK // 16], dt.int16, tag="idx")
                    nc.sync.dma_start(out=it[:, :n // 16], in_=idxP[:, s0 // 16:(s0 + n) // 16])
                    gP = ring.tile([128, (CHUNK // 128) * H], dt.bfloat16, tag="gs")
                    pw = min(WIN, NSROWS - pb)
                    nc.gpsimd.dma_gather(
                        out_ap=gP[:, :nt * H].rearrange("p (k d) -> p k d", d=H),
                        in_ap=P[pb:pb + pw, :], idxs_ap=it[:, :n // 16],
                        num_idxs=n, num_idxs_reg=n, elem_size=H, single_packet=False)
                    ec = ring.tile([64, CHUNK], dt.bfloat16, tag="ea")
                    nc.sync.dma_start(out=ec[:, :n], in_=eaT[:, s0:s0 + n])
                    hstage = stg.tile([128, (CHUNK // 128) * H], dt.bfloat16, tag="hst")
                    for t0 in range(0, nt, 2):
                        pn = min(2, nt - t0)
                        ps = psA.tile([128, 2 * H], dt.float32, tag="main")
                        for j in range(pn):
                            t = t0 + j
                            nc.tensor.matmul(out=ps[:, j * H:(j + 1) * H], lhsT=ec[:, t * 128:(t + 1) * 128],
                                             rhs=w1e_t[:], start=True, stop=True)
                        nc.vector.tensor_tensor(out=ps[:, :pn * H], in0=ps[:, :pn * H],
                                                in1=gP[:, t0 * H:(t0 + pn) * H], op=Alu.add)
                        nc.scalar.activation(hstage[:, t0 * H:(t0 + pn) * H], ps[:, :pn * H], Act.Relu)
                    reg, loc = reg_tensor_base(s0)
                    hbuf = hreg0[reg][0]
                    nc.sync.dma_start(
                        out=hbuf[loc:loc + n, :].rearrange("(t p) d -> p t d", p=128),
                        in_=hstage[:, :nt * H].rearrange("p (t d) -> p t d", d=H))


                for call in (meta["l1_calls"] if on("L1E") else []):
                    if call["slot0"] < O1E:
                        l1_do(call)

                # ---- fold helper ----------------------------------------------
                def fold_pass(src_getter, calls, tiles, idx_param, d_sb, ns_out, nslots):
                    psums = {}
                    ti_global = 0
                    for call in calls:
                        s0, n, sb = call["slot0"], call["n"], call["src_base"]
                        nt = n // 128
                        it = ipool.tile([128, CHUNK // 16], dt.int16, tag="idx")
                        nc.sync.dma_start(out=it[:, :n // 16],
                                          in_=idx_param[:, s0 // 16:(s0 + n) // 16])
                        rows = ring.tile([128, (CHUNK // 128) * H], dt.bfloat16, tag="gs")
                        srct, srcrows = src_getter(sb)
                        pw = min(WIN, srcrows - sb)
                        nc.gpsimd.dma_gather(
                            out_ap=rows[:, :nt * H].rearrange("p (k d) -> p k d", d=H),
                            in_ap=srct[sb:sb + pw, :], idxs_ap=it[:, :n // 16],
                            num_idxs=n, num_idxs_reg=n, elem_size=H, single_packet=False)
                        for t in range(nt):
                            tb = t % 4
                            if tb == 0:
                                nb4 = min(4, nt - t)
                                selb = mw.tile([128, 4 * 128], dt.bfloat16, tag="selb")
                                iota_b = bass.AP(iota_t[:].tensor, iota_t[:].offset,
                                                 [list(iota_t[:].ap[0]), [0, nb4], list(iota_t[:].ap[1])])
                                nc.vector.tensor_tensor(
                                    out=selb[:, :nb4 * 128].rearrange("p (k j) -> p k j", j=128),
                                    in0=d_sb[:, ti_global:ti_global + nb4].to_broadcast([128, nb4, 128]),
                                    in1=iota_b,
                                    op=Alu.is_equal)
                            tm = tiles[ti_global]
                            g = tm["g"]
                            if tm["start"]:
                                psums[g] = psF.tile([128, H], dt.float32, tag=f"fold{g % 4}", name=f"psfold{g % 4}")
                            nc.tensor.matmul(out=psums[g][:], lhsT=selb[:, tb * 128:(tb + 1) * 128],
                                             rhs=rows[:, t * H:(t + 1) * H],
                                             start=tm["start"], stop=tm["stop"])
                            if tm["stop"]:
                                ost = stg.tile([128, H], dt.bfloat16, tag="fst")
                                nc.scalar.activation(ost[:], psums[g][:], Act.Copy)
                                nc.sync.dma_start(out=ns_out[g * 128:(g + 1) * 128, :], in_=ost[:])
                            ti_global += 1

                if on("fold0"):
                    fold_pass(lambda sb: (h0E, O1E), meta["fold_calls"], meta["fold_tiles"],
                              idxF, dF_t, ns0s, meta["foldF_slots"])

                if timing_mode:
                    nc.sync.dma_start(out=ns0f[:S, :], in_=ns0s[:, :])
                else:
                    nc.gpsimd.collective_compute(
                        "AllGather", Alu.bypass, replica_groups=[list(range(NC))],
                        ins=[ns0s[:, :].opt()], outs=[ns0f[:, :].opt()])

                for call in (meta["l1_calls"] if on("L1RD") else []):
                    if call["slot0"] >= O1E:
                        l1_do(call)

                # ---- iter pass helper ------------------------------------------
                TG = 512  # transposed-gather rows per call

                def iter_pass(calls, sfull, hreg, idx_s, idx_g, out_regions, bias):
                    for call in calls:
                        s0, n = call["slot0"], call["n"]
                        nt = n // 128
                        hb = bias(call, nt)
                        hstage = stg.tile([128, (CHUNK // 128) * H], dt.bfloat16, tag="hst")
                        sbase = call["s_base"]
                        sw_rows = min(WIN, NSROWS - sbase)
                        gt, gtrows = hreg[call["g_reg"]]
                        gb = call["g_base"]
                        gw_rows = min(WIN, gtrows - gb)
                        for o in range(0, n, TG):
                            nb = min(TG, n - o)
                            its = ipool.tile([128, TG // 16], dt.int16, tag="idx")
                            nc.sync.dma_start(out=its[:, :nb // 16],
                                              in_=idx_s[:, (s0 + o) // 16:(s0 + o + nb) // 16])
                            itg = ipool.tile([128, TG // 16], dt.int16, tag="idx")
                            nc.sync.dma_start(out=itg[:, :nb // 16],
                                              in_=idx_g[:, (s0 + o) // 16:(s0 + o + nb) // 16])
                            gsT = ring.tile([128, 2 * TG], dt.bfloat16, tag="gsT")
                            nc.gpsimd.dma_gather(
                                out_ap=gsT[:, :2 * nb].rearrange("p (c n) -> p c n", n=nb),
                                in_ap=sfull[sbase:sbase + sw_rows, :], idxs_ap=its[:, :nb // 16],
                                num_idxs=nb, num_idxs_reg=nb, elem_size=H,
                                transpose=True, single_packet=False)
                            ggT = ring.tile([128, 2 * TG], dt.bfloat16, tag="ggT")
                            nc.gpsimd.dma_gather(
                                out_ap=ggT[:, :2 * nb].rearrange("p (c n) -> p c n", n=nb),
                                in_ap=gt[gb:gb + gw_rows, :], idxs_ap=itg[:, :nb // 16],
                                num_idxs=nb, num_idxs_reg=nb, elem_size=H,
                                transpose=True, single_packet=False)
                            mT = mw.tile([128, 2 * TG], dt.bfloat16, tag="mT")
                            nc.vector.tensor_tensor(out=mT[:, :2 * nb], in0=gsT[:, :2 * nb],
                                                    in1=ggT[:, :2 * nb], op=Alu.subtract)
                            nbt = nb // 128
                            for t0 in range(0, nbt, 2):
                                pn = min(2, nbt - t0)
                                ps = psA.tile([128, 2 * H], dt.float32, tag="main")
                                for j in range(pn):
                                    t = t0 + j
                                    nc.tensor.matmul(out=ps[:, j * H:(j + 1) * H],
                                                     lhsT=mT[:, t * 128:t * 128 + 128],
                                                     rhs=w2_t[:, 0:H], start=True, stop=False)
                                    nc.tensor.matmul(out=ps[:, j * H:(j + 1) * H],
                                                     lhsT=mT[:, nb + t * 128:nb + t * 128 + 128],
                                                     rhs=w2_t[:, H:2 * H], start=False, stop=True)
                                tg = (o + t0 * 128) // 128  # tile index within call
                                nc.vector.tensor_tensor(
                                    out=ps[:, :pn * H], in0=ps[:, :pn * H],
                                    in1=hb[:, tg * H:(tg + pn) * H], op=Alu.add)
                                nc.scalar.activation(hstage[:, tg * H:(tg + pn) * H],
                                                     ps[:, :pn * H], Act.Relu)
                        oreg, oloc, obuf = out_regions(call)
                        nc.sync.dma_start(
                            out=obuf[oloc:oloc + n, :].rearrange("(t p) d -> p t d", p=128),
                            in_=hstage[:, :nt * H].rearrange("p (t d) -> p t d", d=H))

                # iter1: bias = sequential h0 read; output h1 regions
                def bias1(call, nt):
                    s0, n = call["slot0"], call["n"]
                    reg, loc = reg_tensor_base(s0)
                    hb = ring.tile([128, (CHUNK // 128) * H], dt.bfloat16, tag="hb")
                    nc.sync.dma_start(
                        out=hb[:, :nt * H].rearrange("p (t d) -> p t d", d=H),
                        in_=hreg0[reg][0][loc:loc + n, :].rearrange("(t p) d -> p t d", p=128))
                    return hb

                def out1(call):
                    s0 = call["slot0"]
                    reg, loc = reg_tensor_base(s0)
                    return reg, loc, hreg1[reg][0]

                if on("it1a"):
                    iter_pass([c for c in meta["it1_calls"] if c["t"] == 0],
                              ns0f, hreg0, idx1s, idx1g, out1, bias1)

                if on("fold1"):
                    fold_pass(lambda sb: (h1E, O1E), meta["fold_calls"], meta["fold_tiles"],
                              idxF, dF_t, ns1s, meta["foldF_slots"])

                if timing_mode:
                    nc.sync.dma_start(out=ns1f[:S, :], in_=ns1s[:, :])
                else:
                    nc.gpsimd.collective_compute(
                        "AllGather", Alu.bypass, replica_groups=[list(range(NC))],
                        ins=[ns1s[:, :].opt()], outs=[ns1f[:, :].opt()])

                if on("it1b"):
                    iter_pass([c for c in meta["it1_calls"] if c["t"] == 1],
                              ns0f, hreg0, idx1s, idx1g, out1, bias1)

                # iter2: bias = gathered h0E rows
                def bias2(call, nt):
                    s0, n = call["slot0"], call["n"]
                    ith = ipool.tile([128, CHUNK // 16], dt.int16, tag="idx")
                    nc.sync.dma_start(out=ith[:, :n // 16],
                                      in_=idx2h[:, s0 // 16:(s0 + n) // 16])
                    hb = ring.tile([128, (CHUNK // 128) * H], dt.bfloat16, tag="hb")
                    hbase = call["h_base"]
                    hw = min(WIN, O1E - hbase)
                    nc.gpsimd.dma_gather(
                        out_ap=hb[:, :nt * H].rearrange("p (k d) -> p k d", d=H),
                        in_ap=h0E[hbase:hbase + hw, :], idxs_ap=ith[:, :n // 16],
                        num_idxs=n, num_idxs_reg=n, elem_size=H, single_packet=False)
                    return hb

                def out2(call):
                    return "2", call["slot0"], h2d

                if on("it2"):
                    iter_pass(meta["it2_calls"], ns1f, hreg1, idx2s, idx2g, out2, bias2)

                if on("fold2"):
                    fold_pass(lambda sb: (h2d, m2pad), meta["fold2_calls"], meta["fold2_tiles"],
                              idxF2, dF2_t, ns2d, meta["foldF2_slots"])

                # ---- L3 + pooling ---------------------------------------------
                # graph-group tile ranges
                gg_first = {}
                gg_last = {}
                for ti in range(NG):
                    gg = tile_gg[ti]
                    if gg not in gg_first:
                        gg_first[gg] = ti
                    gg_last[gg] = ti
                XC = 2048
                psG = None
                for b in range(0, S, XC) if on("L3") else []:
                    nb_ = min(XC, S - b)
                    xc = ring.tile([128, XC], dt.bfloat16, tag="xc")
                    nc.sync.dma_start(out=xc[:, :nb_], in_=xT_c[:, b:b + nb_])
                    for t in range(nb_ // 128):
                        ti = (b + t * 128) // 128
                        v = ring.tile([128, H], dt.bfloat16, tag="v")
                        nc.sync.dma_start(out=v[:], in_=ns2d[ti * 128:(ti + 1) * 128, :])
                        vT = mw.tile([128, H], dt.bfloat16, tag="mT")
                        for k in range(2):
                            pt = psT.tile([128, 128], dt.bfloat16, tag="tr")
                            nc.tensor.transpose(out=pt[:], in_=v[:, k * 128:(k + 1) * 128],
                                                identity=ident_t[:])
                            nc.vector.tensor_copy(out=vT[:, k * 128:(k + 1) * 128], in_=pt[:])
                        ps = psA.tile([128, H], dt.float32, tag="main")
                        nc.tensor.matmul(out=ps[:], lhsT=xc[:, t * 128:(t + 1) * 128],
                                         rhs=w3x_t[:], start=True, stop=False)
                        nc.tensor.matmul(out=ps[:], lhsT=vT[:, 0:128], rhs=w3v_t[:, 0:H],
                                         start=False, stop=False)
                        nc.tensor.matmul(out=ps[:], lhsT=vT[:, 128:256], rhs=w3v_t[:, H:2 * H],
                                         start=False, stop=False)
                        nc.tensor.matmul(out=ps[:], lhsT=ones_t[:], rhs=b3_t[:],
                                         start=False, stop=True)
                        na = mw.tile([128, H], dt.bfloat16, tag="na")
                        nc.scalar.activation(na[:], ps[:], Act.Relu)
                        # pool
                        gg = tile_gg[ti]
                        sel = mw.tile([128, 128], dt.bfloat16, tag="sel")
                        nc.vector.tensor_tensor(
                            out=sel[:], in0=dPool_t[:, ti:ti + 1].to_broadcast([128, 128]),
                            in1=iota_t[:], op=Alu.is_equal)
                        if ti == gg_first[gg]:
                            psG = psF.tile([128, H], dt.float32, tag="fold0")
                        nc.tensor.matmul(out=psG[:], lhsT=sel[:], rhs=na[:],
                                         start=(ti == gg_first[gg]), stop=(ti == gg_last[gg]))
                        if ti == gg_last[gg]:
                            ostage = stg.tile([128, H], dt.float32, tag="ost")
                            nc.vector.tensor_copy(out=ostage[:], in_=psG[:])
                            nc.sync.dma_start(out=out[gg * 128:(gg + 1) * 128, :], in_=ostage[:])

    nc.compile()
    return nc



import time
import jax
from jax.sharding import Mesh, PartitionSpec
from jax.experimental.shard_map import shard_map
from concourse import bass2jax
from concourse.bass2jax import _bass_exec_p, install_neuronx_cc_hook



def make_runner(nc, in_maps, n_cores=8):
    install_neuronx_cc_hook()
    partition_name = nc.partition_id_tensor.name if nc.partition_id_tensor else None
    in_names, out_names, out_avals, zero_outs = [], [], [], []
    for alloc in nc.m.functions[0].allocations:
        if not isinstance(alloc, mybir.MemoryLocationSet):
            continue
        name = alloc.memorylocations[0].name
        if alloc.kind == "ExternalInput":
            if name != partition_name:
                in_names.append(name)
        elif alloc.kind == "ExternalOutput":
            out_names.append(name)
            shape = tuple(alloc.tensor_shape)
            dtype = mybir.dt.np(alloc.dtype)
            out_avals.append(jax.core.ShapedArray(shape, dtype))
            zero_outs.append(np.zeros(shape, dtype))
    n_params = len(in_names)
    all_in = list(in_names) + list(out_names)
    if partition_name is not None:
        all_in.append(partition_name)

    def _body(*args):
        operands = list(args)
        if partition_name is not None:
            operands.append(bass2jax.partition_id_tensor())
        outs = _bass_exec_p.bind(
            *operands,
            out_avals=tuple(out_avals),
            in_names=tuple(all_in),
            out_names=tuple(out_names),
            lowering_input_output_aliases=(),
            sim_require_finite=True,
            sim_require_nnan=True,
            nc=nc,
        )
        return tuple(outs)

    devices = jax.devices()[:n_cores]
    mesh = Mesh(np.asarray(devices), ("core",))
    n_outs = len(out_names)
    in_specs = (PartitionSpec("core"),) * (n_params + n_outs)
    out_specs = (PartitionSpec("core"),) * n_outs
    fn = jax.jit(shard_map(_body, mesh=mesh, in_specs=in_specs,
                           out_specs=out_specs, check_rep=False), keep_unused=True)

    sharding = jax.sharding.NamedSharding(mesh, PartitionSpec("core"))
    dev_in = []
    for i, name in enumerate(in_names):
        cat = np.concatenate([np.asarray(in_maps[c][name]) for c in range(n_cores)], axis=0)
        dev_in.append(jax.device_put(cat, sharding))
    for z in zero_outs:
        cat = np.zeros((n_cores * z.shape[0], *z.shape[1:]), z.dtype)
        dev_in.append(jax.device_put(cat, sharding))

    def run():
        outs = fn(*dev_in)
        jax.block_until_ready(outs)
        return outs

    def results(outs):
        return [
            {name: np.asarray(outs[i]).reshape(n_cores, *out_avals[i].shape)[c]
             for i, name in enumerate(out_names)}
            for c in range(n_cores)
        ]
    return run, results


def time_runner(run, iters=3):
    run()  # compile + warm
    ts = []
    for _ in range(iters):
        t0 = time.perf_counter()
        run()
        ts.append(time.perf_counter() - t0)
    return min(ts), ts


def kernel(**inputs):
    in_maps, meta, post = preprocess(inputs, NUM_GRAPHS, NC=8, WIN=32768, CHUNK=2048)
    nc = build(meta)
    run, results = make_runner(nc, in_maps, 8)
    res = results(run())
    G = NUM_GRAPHS
    full = np.zeros((G, 256), np.float32)
    for c in range(8):
        glo, ghi = post["gr"][c]
        full[glo:ghi] = res[c]["out"][:ghi - glo]
    return full

